# revision 36
# baseline (speedup 1.0000x reference)
"""Trainium2 Bass kernel for nn_MultiHeadAttention_54614804136658.

Forward pass of the reference collapses to: out = v + sum_h P_h[argmax_j(qh_h @ kh_h^T)]
where P_h = v @ (w_vs_h @ w_fc_h), because the straight-through estimator
(hard - stop_grad(attn) + attn) makes the forward attention an exact one-hot of
the score argmax (softmax/topk/scale are monotonic and keep the max).

Sharding: 8 cores = 2 batches x 4 head-groups (2 heads each). Per core:
  A: khT fp32 projection streamed per kt column-chunk DMA; qhT col 0 projected
     up-front, cols 1-3 one accumulation-matmul per steady tile
  B: P = v @ [W_h0|W_h1] in bf16, one row-tile per steady tile, ACT evacuates
     -> pscr0/pscr1 (DRAM, bf16)
  steady loop over 32 (h,t) score tiles (DVE-bound at ~4.5us/tile):
    PE: 2 half-tile fp32 matmuls [128,1024] into PSUM (+1 qhT/B matmul)
    ACT: PSUM -> SBUF copy
    DVE: max8 + max_index over [128,2048] SBUF
    Pool: per-tile indirect row-gather of P rows by the argmax indices
          (per-partition SBUF offsets -- no DRAM index roundtrip)
    SP: batched g -> out DMA every 4 tiles
Host: fuses W = w_vs_h @ w_fc_h, transposes/slices inputs, sums partials + v.
"""
import numpy as np
from contextlib import ExitStack

B, L, E = 2, 2048, 512
H, DQK, DV = 8, 64, 256
QT = L // 128           # 16 query tiles
ETIL = E // 128         # 4 embed tiles

_CACHE = {}


def _build(phases="ABCD", num_devices=8):
    import concourse.bass as bass
    import concourse.tile as tile
    from concourse import bacc, mybir

    F32 = mybir.dt.float32
    BF16 = mybir.dt.bfloat16
    U32 = mybir.dt.uint32
    I32 = mybir.dt.int32
    OP = mybir.AluOpType
    AX = mybir.AxisListType

    nc = bacc.Bacc("TRN2", target_bir_lowering=False, debug=False,
                   num_devices=num_devices)
    dbg = num_devices == 1

    qt_d = nc.dram_tensor("qt", [E, L], F32, kind="ExternalInput").ap()
    kt_d = nc.dram_tensor("kt", [E, L], F32, kind="ExternalInput").ap()
    vt_d = nc.dram_tensor("vt", [DV, L], BF16, kind="ExternalInput").ap()
    wq_d = nc.dram_tensor("wq", [E, 128], F32, kind="ExternalInput").ap()
    wk_d = nc.dram_tensor("wk", [E, 128], F32, kind="ExternalInput").ap()
    W_d = nc.dram_tensor("W", [2, DV, DV], BF16, kind="ExternalInput").ap()
    rr_d = nc.dram_tensor("rr", [1, L], mybir.dt.int32, kind="ExternalInput").ap()
    out_d = nc.dram_tensor("out", [2, L, DV], BF16, kind="ExternalOutput").ap()
    pscr = [nc.dram_tensor(f"pscr{h}", [L, DV], BF16,
                           kind="ExternalOutput" if dbg else "Internal").ap()
            for h in range(2)]

    with tile.TileContext(nc) as tc, ExitStack() as ctx:
        keep = ctx.enter_context(tc.tile_pool(name="keep", bufs=1))
        qhT = keep.tile([128, L], F32, tag="qhT")   # 2 heads stacked 64+64
        khT = keep.tile([128, L], F32, tag="khT")
        P_s = keep.tile([128, QT, 2, DV], BF16, tag="P")

        # ---------- input DMAs: large transfers, 3 queues, kt first ----------
        ldQ = ctx.enter_context(tc.tile_pool(name="ldQ", bufs=1))
        ldB = ctx.enter_context(tc.tile_pool(name="ldB", bufs=1))
        ldK_cm = tc.tile_pool(name="ldK", bufs=1)
        ldK = ldK_cm.__enter__()
        wk_s = ldK.tile([128, ETIL, 128], F32, tag="wk")
        nc.sync.dma_start(wk_s[:], wk_d.rearrange("(t p) m -> p t m", p=128))
        wq_s = ldQ.tile([128, ETIL, 128], F32, tag="wq")
        nc.scalar.dma_start(wq_s[:], wq_d.rearrange("(t p) m -> p t m", p=128))
        kt_s = ldK.tile([128, ETIL, L], F32, tag="kt")
        qt_s = ldQ.tile([128, ETIL, L], F32, tag="qt")
        Q3 = [nc.sync, nc.scalar, nc.gpsimd]
        for cc in range(8):   # kt column-pieces: khT col-group ready per piece
            Q3[cc % 3].dma_start(
                kt_s[:, :, cc * 256:(cc + 1) * 256],
                kt_d[:, cc * 256:(cc + 1) * 256]
                .rearrange("(t p) c -> p t c", p=128))
        nc.gpsimd.dma_start(
            qt_s[:, :, 0:512], qt_d[:, 0:512].rearrange("(t p) c -> p t c", p=128))
        vt_s = ldB.tile([128, 2, L], BF16, tag="vt")
        nc.sync.dma_start(vt_s[:], vt_d.rearrange("(t p) n -> p t n", p=128))
        W_s = ldB.tile([128, 2, 2, DV], BF16, tag="W")
        for h in range(2):
            nc.scalar.dma_start(W_s[:, :, h, :],
                                W_d[h].rearrange("(t p) m -> p t m", p=128))
        for cc in range(1, 4):
            Q3[cc % 3].dma_start(
                qt_s[:, :, cc * 512:(cc + 1) * 512],
                qt_d[:, cc * 512:(cc + 1) * 512]
                .rearrange("(t p) c -> p t c", p=128))
        rr1 = ldB.tile([1, L], I32, tag="rr1")
        nc.scalar.dma_start(rr1[:], rr_d)
        rrow = ldB.tile([128, L], I32, tag="rrow")
        nc.gpsimd.partition_broadcast(rrow[:], rr1[:])
        cst = ldB.tile([128, 2], U32, tag="cst")
        nc.vector.memset(cst[:, 0:1], 7)
        nc.vector.memset(cst[:, 1:2], 3)

        # ---------- phase A: khT streamed per kt column chunk ----------
        with tc.tile_pool(name="psA", bufs=1, space="PSUM") as psA:
            pss = [psA.tile([128, 512], F32, tag=f"psA{nb}", name=f"psA{nb}")
                   for nb in range(4)]
            for cg in range(8):
                for et in range(ETIL):
                    nc.tensor.matmul(
                        pss[cg // 2][:, (cg % 2) * 256:(cg % 2 + 1) * 256],
                        wk_s[:, et, :],
                        kt_s[:, et, cg * 256:(cg + 1) * 256],
                        start=(et == 0), stop=(et == ETIL - 1))
                if cg % 2 == 1:
                    nc.scalar.copy(khT[:, (cg - 1) * 256:(cg + 1) * 256],
                                   pss[cg // 2][:])
        ldK_cm.__exit__(None, None, None)

        # ---------- steady loop ----------
        if "C" in phases:
          with tc.tile_pool(name="scps", bufs=2, space="PSUM") as scps, \
               tc.tile_pool(name="psQ", bufs=1, space="PSUM") as psQ, \
               tc.tile_pool(name="psB", bufs=2, space="PSUM") as psB, \
               tc.tile_pool(name="ysb", bufs=4) as ysb, \
               tc.tile_pool(name="fsb", bufs=2) as fsb, \
               tc.tile_pool(name="bmsb", bufs=3) as bmsb, \
               tc.tile_pool(name="xsb", bufs=2) as xsb, \
               tc.tile_pool(name="scsb", bufs=20) as scsb, \
               tc.tile_pool(name="gsb", bufs=1) as gsb:

            psq_cur = [None]

            def project_q_mm(cg, et):
                # cg indexes 256-column groups (8 of them)
                if et == 0:
                    psq_cur[0] = psQ.tile([128, 256], F32, tag="q", name="ps_q")
                psq = psq_cur[0]
                nc.tensor.matmul(
                    psq[:], wq_s[:, et, :],
                    qt_s[:, et, cg * 256:(cg + 1) * 256],
                    start=(et == 0), stop=(et == ETIL - 1))
                if et == ETIL - 1:
                    nc.scalar.copy(qhT[:, cg * 256:(cg + 1) * 256], psq[:])

            def b_unit(rt):
                psb = psB.tile([128, 2 * DV], F32, tag="pb", name="ps_pb")
                for et in range(2):
                    nc.tensor.matmul(
                        psb[:], vt_s[:, et, rt * 128:(rt + 1) * 128],
                        W_s[:, et, :, :], start=(et == 0), stop=(et == 1))
                nc.scalar.copy(P_s[:, rt, :, :], psb[:])
                if rt == QT - 1:
                    for h in range(2):
                        nc.scalar.dma_start(
                            pscr[h].rearrange("(t p) e -> p t e", p=128),
                            P_s[:, :, h, :])

            g_cur = {}
            pend = []
            pscr_ready = [False]

            def do_gather(h, t, i8):
                # gather P_h rows by per-partition argmax indices
                if t % 4 == 0:
                    g_cur[h, t // 4] = gsb.tile([128, 4, DV], BF16,
                                                tag=f"g{h}{t // 4}",
                                                name=f"g{h}_{t // 4}")
                g = g_cur[h, t // 4]
                nc.gpsimd.indirect_dma_start(
                    out=g[:, t % 4, :], out_offset=None,
                    in_=pscr[h][:],
                    in_offset=bass.IndirectOffsetOnAxis(ap=i8[:, 0:1], axis=0))
                if t % 4 == 3:
                    nc.sync.dma_start(
                        out_d[h].rearrange("(t p) e -> p t e", p=128)
                        [:, t - 3:t + 1, :], g[:])

            PACKED = ()
            LAG = 3   # packed DVE ops emit 3 tiles late: hides the
                      # PE->ACT->Pool->Pool chain from the in-order DVE queue
            dve_defer = []

            def packed_dve(h, t, X):
                # one-scan argmax on the int-packed array
                BM = bmsb.tile([128, 256], I32, tag="BM")
                nc.vector.tensor_reduce(
                    BM[:].rearrange("p (a b) -> p a b", a=2),
                    X[:].rearrange("p (a c b) -> p a c b", a=2, b=8),
                    AX.X, OP.max)
                m8i = scsb.tile([128, 8], I32, tag="m8i")
                nc.vector.max(m8i[:], BM[:])
                i8p = scsb.tile([128, 8], U32, tag="i8p")
                nc.vector.max_index(i8p[:], m8i[:], BM[:])
                r1t = scsb.tile([128, 1], U32, tag="r1t")
                nc.vector.scalar_tensor_tensor(
                    r1t[:], m8i[:, 0:1].bitcast(U32), cst[:, 0:1],
                    cst[:, 0:1], op0=OP.bitwise_and, op1=OP.bitwise_xor)
                kk = scsb.tile([128, 1], U32, tag="kk")
                nc.vector.scalar_tensor_tensor(
                    kk[:], i8p[:, 0:1], cst[:, 1:2], r1t[:],
                    op0=OP.logical_shift_left, op1=OP.bitwise_or)
                pend.append((h, t, kk))

            def drain_defer(now_j, force=False):
                while dve_defer and (force or dve_defer[0][0] <= now_j - LAG):
                    _, hh, tt, X = dve_defer.pop(0)
                    packed_dve(hh, tt, X)

            def drain_gathers(j, force=False):
                if j >= QT - 1:
                    n = len(pend) if force else (2 if len(pend) > 8 else 1)
                    for (hh, tt, ii) in pend[:n]:
                        do_gather(hh, tt, ii)
                    del pend[:n]

            for et in range(ETIL):
                project_q_mm(0, et)
            for h in range(2):
                for t in range(QT):
                    if h == 0 and t < 14:
                        # 2 accumulation-mms per tile: group cg done by
                        # tile 2cg-1, needed first at tile 2cg
                        for u in range(2):
                            uu = 2 * t + u
                            project_q_mm(1 + uu // 4, uu % 4)
                    j = h * QT + t
                    packed = j in PACKED
                    drain_defer(j)
                    if packed:
                        y = fsb.tile([128, L], I32, tag="F", name="F")
                    else:
                        y = ysb.tile([128, L], F32, tag="y", name="y")
                    for half in range(2):
                        ps = scps.tile([128, 1024], F32, tag="sc", name="ps_sc")
                        for kb in range(2):
                            col = half * 1024 + kb * 512
                            nc.tensor.matmul(
                                ps[:, kb * 512:(kb + 1) * 512],
                                qhT[h * 64:(h + 1) * 64, t * 128:(t + 1) * 128],
                                khT[h * 64:(h + 1) * 64, col:col + 512],
                                start=True, stop=True)
                        if packed:
                            # ACT: F = round(S * 2^15) as int32
                            nc.scalar.activation(
                                y[:, half * 1024:(half + 1) * 1024], ps[:],
                                mybir.ActivationFunctionType.Copy,
                                bias=0.0, scale=float(2 ** 15))
                        else:
                            nc.scalar.copy(
                                y[:, half * 1024:(half + 1) * 1024], ps[:])
                    if h == 0 and 8 <= t < 16:
                        b_unit(2 * (t - 8))
                        b_unit(2 * (t - 8) + 1)
                    if packed:
                        G = xsb.tile([128, L], I32, tag="G", name="G")
                        nc.gpsimd.tensor_scalar(G[:], y[:], 8, None, op0=OP.mult)
                        X = xsb.tile([128, L], I32, tag="X", name="X")
                        nc.gpsimd.tensor_tensor(X[:], G[:], rrow[:], op=OP.add)
                        dve_defer.append((j, h, t, X))
                    else:
                        m8 = scsb.tile([128, 8], F32, tag="m8")
                        nc.vector.max(m8[:], y[:])
                        i8 = scsb.tile([128, 8], U32, tag="i8")
                        nc.vector.max_index(i8[:], m8[:], y[:])
                        pend.append((h, t, i8))
                    if "D" in phases:
                        drain_gathers(j)
            drain_defer(0, force=True)
            if "D" in phases:
                drain_gathers(2 * QT - 1, force=True)

    nc.compile()
    return nc


def kernel(**inputs):
    from concourse.bass_utils import run_bass_kernel_spmd

    q = np.asarray(inputs["q"], np.float32)
    k = np.asarray(inputs["k"], np.float32)
    v = np.asarray(inputs["v"], np.float32)
    w_qs = np.asarray(inputs["w_qs"], np.float32)
    w_ks = np.asarray(inputs["w_ks"], np.float32)
    w_vs = np.asarray(inputs["w_vs"], np.float32)
    w_fc = np.asarray(inputs["w_fc"], np.float32)

    if "nc" not in _CACHE:
        _CACHE["nc"] = _build()
    nc = _CACHE["nc"]

    import ml_dtypes
    bf16 = ml_dtypes.bfloat16

    # fused per-head value->output projection
    W = np.empty((H, DV, DV), np.float32)
    for h in range(H):
        W[h] = (w_vs[:, h * DV:(h + 1) * DV].astype(np.float64)
                @ w_fc[h * DV:(h + 1) * DV, :].astype(np.float64)).astype(np.float32)

    in_maps = []
    for c in range(8):
        b, g = divmod(c, 4)
        in_maps.append({
            "qt": np.ascontiguousarray(q[b].T),
            "kt": np.ascontiguousarray(k[b].T),
            "vt": np.ascontiguousarray(v[b].T).astype(bf16),
            "wq": np.ascontiguousarray(w_qs[:, g * 128:(g + 1) * 128]),
            "wk": np.ascontiguousarray(w_ks[:, g * 128:(g + 1) * 128]),
            "W": np.ascontiguousarray(W[2 * g:2 * g + 2]).astype(bf16),
            "rr": (7 - (np.arange(L) & 7)).astype(np.int32)[None, :],
        })

    res = run_bass_kernel_spmd(nc, in_maps, core_ids=list(range(8)))
    _CACHE["last_result"] = res

    out = np.array(v)  # residual
    for c in range(8):
        b = c // 4
        co = res.results[c]["out"]
        out[b] += np.asarray(co[0], np.float32)
        out[b] += np.asarray(co[1], np.float32)
    return out
